# revision 1
# baseline (speedup 1.0000x reference)
"""Trainium2 Bass kernel for a RoPE causal-attention layer (v2, bf16).

Problem (hardcoded): B=2, T=2048, DIM=1024, H=16 heads, Dh=64, fp32 ref.
  qkv = x @ qkv_w.T + qkv_b ; rope(q), rope(k) ; causal softmax(q k^T/sqrt(Dh)) @ v
  out = ctx @ out_w.T + out_b

Sharding: tensor-parallel over heads - each of the 8 cores owns 2 heads
(qkv_w row-sharded, out_w column-sharded); per-core partial outputs are
summed on the host.

v2 design (vs the fp32r v1 baseline, 225us):
  * Everything stored bf16 (SBUF + DMA); PSUM accumulation stays f32.
    End-to-end rel err ~4e-3 (tolerance 2e-2). Halves DMA bytes, gives
    DVE its 4x bf16 mode; PE rate is the same as fp32r but with no
    <256-column penalty.
  * RoPE's half-swap is done by partition-OFFSET DVE multiplies (engines
    accept mismatched partition offsets between operands), so there is
    no PE swap matmul and no DMA bounce in the rope chain.
  * V is projected directly TRANSPOSED ([token, feature] blocks) by
    putting the x-tile as the stationary operand, so there are no PE
    transposes; v bias is dropped on device (softmax rows sum to 1 so
    Wo @ bv is added to out_b on the host).
  * Scores for BOTH heads land in one [128, 1024] 2-bank PSUM tile and
    are exp'd by a single fused Activation op per key-block.
  * The softmax denominator comes from a ones-column appended to V; the
    reciprocal row is broadcast across partitions with a PE ones-matmul
    (bounced via Act to SBUF so the normalize muls have only one PSUM
    operand - the walrus verifier's TensorTensor constraint).
  * Output projection partials are evicted PSUM->bf16 SBUF on DVE/Act
    (Pool cannot read PSUM, DMA cannot source from PSUM) and stored with
    two half-chunk DMAs per 512-token chunk.
  * Emission-order software pipelining: attention (Act-heavy) for block
    i is woven step-by-step with projections/output-proj (PE-heavy) of
    other chunks so the PE never waits on exp and the Activation engine
    never starves the PE. Emission order is each engine queue's program
    order, so per-phase PE work is emitted before any later DMA enters
    the queues it waits on.
"""

import sys

if "/opt/trn_rl_repo" not in sys.path:
    sys.path.insert(0, "/opt/trn_rl_repo")

import numpy as np
import ml_dtypes

import concourse.bass as bass
import concourse.tile as tile
from concourse import mybir
from concourse.vector_clock import ScopedClock, VectorClock

B, T, DIM = 2, 2048, 1024
H, Dh = 16, 64
NCORES = 8
HPC = H // NCORES          # heads per core
NT = B * T                 # 4096 tokens
RPC = HPC * Dh             # 128 rows per core for each of q/k/v
NCH = NT // 512            # 8 column chunks
SCALE = Dh ** -0.5

F32 = mybir.dt.float32
BF = mybir.dt.bfloat16
NPBF = ml_dtypes.bfloat16


def _patch_tile_drain():
    """This container's walrus build allows at most ONE semaphore wait per
    instruction (setupSyncWait rejects more).  Two fixes:
      1. Tile's end-of-kernel drain accumulates one wait per live
         semaphore - split into one drain per semaphore.
      2. Any scheduled instruction that received >1 sem waits in stage 1B
         gets its extra waits hoisted onto same-engine NoOps emitted just
         before it during lowering."""
    if getattr(tile.TileContext, "_drain_patched", False):
        return

    def patched(self, tick_clock, wait_clock):
        vec = list(tick_clock.global_clock)
        nz = [(i, t) for i, t in enumerate(vec) if t > 0] or [(0, 0)]
        # distribute the per-sem drain waits across engine queues so they
        # park in parallel instead of serializing on the sync queue
        engs = [self.nc.sync, self.nc.scalar, self.nc.vector,
                self.nc.gpsimd, self.nc.tensor]
        for k, (i, t) in enumerate(nz):
            cv = [0] * len(vec)
            cv[i] = t
            d = engs[k % len(engs)].drain()
            wait_clock.add_sem_waits(d.ins, ScopedClock({None: VectorClock(cv)}))
        self.nc.all_engine_barrier()
        popped = self.nc._tile_sem_poison_stack.pop()
        assert popped is self._sem_poison
        self.nc.clear_and_free_semaphores(list(self.sems.allocated().values()))
        self.nc.all_engine_barrier()

    tile.TileContext._drain_and_barrier = patched

    orig_cal = tile.TileContext._commit_and_lower

    def patched_cal(self, inst, original_block, old_bb_map, bb_to_exit_bb):
        si = getattr(inst, "sync_info", None)
        eng = getattr(inst, "engine", None)
        if si is not None and si.on_wait and eng in self.nc.engines:
            waits = list(si.on_wait)
            keep = 1
            if len(waits) > keep:
                for w in waits[: len(waits) - keep]:
                    nop = self.nc.engines[eng].nop(nofuse=True)
                    nop.ins.sync_info = mybir.SyncInfo(on_wait=[w], on_update=[])
                inst.sync_info = mybir.SyncInfo(
                    on_wait=waits[len(waits) - keep:],
                    on_update=list(si.on_update or []),
                )
        return orig_cal(self, inst, original_block, old_bb_map, bb_to_exit_bb)

    tile.TileContext._commit_and_lower = patched_cal
    tile.TileContext._drain_patched = True


SHUF_MASK = [(i + 16) % 32 for i in range(32)]


def _rope_tables():
    """C, S [128, 2048] (f32) for the quadrant-local q/k row layout, rows
    duplicated for the 2 heads resident on a core.

    Row layout per 64-row head (see _qk_row_perm): each 32-partition
    quadrant holds [e_p... , o_p...] for 16 pairs p, so the RoPE partner of
    every row lives 16 partitions away WITHIN its quadrant - exactly what
    one DVE stream_shuffle with a rotate-16 mask delivers:
      q_rot = q*C + shuffle16(q)*S,
      C[r] = cos(angle_{pair(r)}), S[r] = -sin if r is an 'e' row else +sin.
    """
    half = Dh // 2  # 32
    inv_freq = 1.0 / 10000.0 ** (np.arange(half, dtype=np.float64) / half)
    t = np.arange(T, dtype=np.float64)
    pair_freq = inv_freq[(2 * np.arange(half)) % half]        # [32] per pair
    C64 = np.empty((Dh, T))
    S64 = np.empty((Dh, T))
    for r in range(Dh):
        q, i = divmod(r, 32)
        p = q * 16 + (i % 16)
        ang = pair_freq[p] * t
        C64[r] = np.cos(ang)
        S64[r] = -np.sin(ang) if i < 16 else np.sin(ang)
    C = np.concatenate([C64, C64], axis=0).astype(np.float32)  # [128, T]
    S = np.concatenate([S64, S64], axis=0).astype(np.float32)
    return np.ascontiguousarray(C), np.ascontiguousarray(S)


def _qk_row_perm():
    """perm such that new[r] = old[perm[r]] for one head's 64 rows:
    quadrant q (rows 32q..32q+31) holds evens of pairs 16q..16q+15 then
    odds of the same pairs."""
    perm = np.empty(Dh, dtype=np.int64)
    for q in range(2):
        for i in range(16):
            perm[q * 32 + i] = 2 * (q * 16 + i)
            perm[q * 32 + 16 + i] = 2 * (q * 16 + i) + 1
    return perm


def _tri_mask():
    """[128, 128] mask[i, j] = 1 if i <= j else 0 (transposed-score diag)."""
    i = np.arange(128)[:, None]
    j = np.arange(128)[None, :]
    return (i <= j).astype(np.float32)


def _weave(primary, filler, front=1.0):
    """Distribute filler steps evenly among the first `front` fraction of
    primary steps (front<1 keeps the engine queues clear near the end of
    the primary phase, e.g. before its tail chain)."""
    out = []
    npr, nf = len(primary), len(filler)
    span = max(1, int(npr * front))
    fi = 0
    for k, p in enumerate(primary):
        out.append(p)
        want = min(nf, ((k + 1) * nf) // span)
        while fi < want:
            out.append(filler[fi])
            fi += 1
    out.extend(filler[fi:])
    return out


# schedule configuration (tuned via timeline-sim sweep)
CFG = {
    "plan": "P1",          # which superstep/filler layout
    "evict": "ddadddad",   # engine per out-proj e-block (d=DVE a=Act)
    "cu_eng": "d",         # context-eviction engine (d=DVE a=Act)
    "norm": "pe",          # softmax normalize: "pe" bcast-matmul | "dma"
    "xbufs": 3,
    "exbufs": 5,
}


def _build_nc(use_pad_mask: bool):
    _patch_tile_drain()
    nc = bass.Bass("TRN2", target_bir_lowering=False, debug=False,
                   num_devices=NCORES)

    xT8 = nc.dram_tensor("xT8", [128, 8, NT], BF, kind="ExternalInput")
    # constants are packed into 4 fused tensors - each HWDGE issue costs
    # ~1.3us of queue time, so fewer/bigger const DMAs matter at startup
    wqkv = nc.dram_tensor("wqkv", [128, 3, 8, 128], BF, kind="ExternalInput")
    bqk = nc.dram_tensor("bqk", [RPC, 2], F32, kind="ExternalInput")
    ropecs = nc.dram_tensor("ropecs", [RPC, 2, T], BF, kind="ExternalInput")
    wotri = nc.dram_tensor("wotri", [128, 9, 128], BF, kind="ExternalInput")
    if use_pad_mask:
        padv = nc.dram_tensor("padv", [B, 128, T // 128], F32,
                              kind="ExternalInput")
    out8 = nc.dram_tensor("out8", [128, 8, NT], BF, kind="ExternalOutput")

    EXP = mybir.ActivationFunctionType.Exp
    IDN = mybir.ActivationFunctionType.Identity
    CPY = mybir.ActivationFunctionType.Copy

    with tile.TileContext(nc) as tc:
        with (
            tc.tile_pool(name="consts", bufs=1) as consts,
            tc.tile_pool(name="persist", bufs=1) as persist,
            tc.tile_pool(name="xpool", bufs=CFG.get("xbufs", 2)) as xpool,
            tc.tile_pool(name="qkvtmp", bufs=4) as qkvtmp,
            tc.tile_pool(name="ropetmp", bufs=4) as ropetmp,
            tc.tile_pool(name="expool", bufs=CFG.get("exbufs", 3)) as expool,
            tc.tile_pool(name="normtmp", bufs=2) as normtmp,
            tc.tile_pool(name="outstage", bufs=2) as outstage,
            tc.tile_pool(name="drampool", bufs=2, space="DRAM") as drampool,
            tc.tile_pool(name="psS", bufs=2, space="PSUM") as psS,
            tc.tile_pool(name="psC", bufs=1, space="PSUM") as psC,
            tc.tile_pool(name="psP", bufs=2, space="PSUM") as psP,
        ):
            # ---- constants ------------------------------------------------
            wqkv_s = consts.tile([128, 3, 8, 128], BF, tag="wqkv")
            bqk_s = consts.tile([RPC, 2], F32, tag="bqk")
            ropecs_s = consts.tile([RPC, 2, T], BF, tag="ropecs")
            wotri_s = consts.tile([128, 9, 128], BF, tag="wotri")
            ones_s = consts.tile([1, Dh], BF, tag="ones")
            if use_pad_mask:
                pad_s = consts.tile([128, B * (T // 128)], F32, tag="padv")
            def emit_consts_a1():
                nc.scalar.dma_start(out=wqkv_s[:, 0:1, 0:4],
                                    in_=wqkv[:, 0:1, 0:4])

            def emit_consts_a1second():
                nc.scalar.dma_start(out=wqkv_s[:, 0:1, 4:8],
                                    in_=wqkv[:, 0:1, 4:8])
                nc.scalar.dma_start(out=bqk_s[:], in_=bqk[:])

            def emit_consts_a1b():
                nc.scalar.dma_start(out=wqkv_s[:, 1:2], in_=wqkv[:, 1:2])
                nc.scalar.dma_start(out=wqkv_s[:, 2:3], in_=wqkv[:, 2:3])

            def emit_consts_a2():
                # chunk n only reads table cols [512(n%4), +512) - load the
                # first quarter now, the rest after A1's x-tile is queued
                nc.scalar.dma_start(out=ropecs_s[:, :, 0:512],
                                    in_=ropecs[:, :, 0:512])
                nc.vector.memset(ones_s[:], 1.0)

            def emit_consts_a3():
                nc.scalar.dma_start(out=ropecs_s[:, :, 512:T],
                                    in_=ropecs[:, :, 512:T])

            def emit_consts_b():
                nc.scalar.dma_start(out=wotri_s[:], in_=wotri[:])
                if use_pad_mask:
                    for b in range(B):
                        nc.scalar.dma_start(
                            out=pad_s[:, b * 16:(b + 1) * 16], in_=padv[b])

            # ---- persistent activations ----------------------------------
            qrot = [persist.tile([RPC, 512], BF, tag=f"qrot{n}",
                                 name=f"qrot{n}") for n in range(NCH)]
            krot = [persist.tile([RPC, 512], BF, tag=f"krot{n}",
                                 name=f"krot{n}") for n in range(NCH)]
            ctxt = [persist.tile([RPC, 512], BF, tag=f"ctxt{n}",
                                 name=f"ctxt{n}") for n in range(NCH)]
            vaug = {}

            def emit_vaug_init():
                for b in range(B):
                    va = persist.tile([128, HPC, 16, 65], BF, tag=f"vaug{b}",
                                      name=f"vaug{b}")
                    nc.vector.memset(va[:], 1.0)
                    vaug[b] = va

            # ---- phase A: QKV projection + rope for one 512-token chunk --
            def a_steps(n, split_load=False):
                t0 = n * 512
                tl = t0 % T
                b = t0 // T
                blk0 = 4 * (n % 4)
                st = {}

                def xload():
                    xt = xpool.tile([128, 8, 512], BF, tag="xt",
                                    name=f"xt{n}")
                    if split_load:
                        # subtile deps let the first 4 K-chunks of the
                        # projection start as soon as the first half lands
                        nc.sync.dma_start(out=xt[:, 0:4, :],
                                          in_=xT8[:, 0:4, t0:t0 + 512])
                        nc.sync.dma_start(out=xt[:, 4:8, :],
                                          in_=xT8[:, 4:8, t0:t0 + 512])
                    else:
                        nc.sync.dma_start(out=xt[:],
                                          in_=xT8[:, :, t0:t0 + 512])
                    st["xt"] = xt

                def proj(widx, key):
                    def f():
                        ps = psP.tile([128, 512], F32, tag="proj",
                                      name=f"{key}ps{n}")
                        xt = st["xt"]
                        for kc in range(8):
                            nc.tensor.matmul(ps[:], wqkv_s[:, widx, kc, :],
                                             xt[:, kc, :], start=(kc == 0),
                                             stop=(kc == 7))
                        st[key] = ps
                    return f

                def post(key, bidx, dst):
                    def f():
                        ps = st.pop(key)
                        raw = qkvtmp.tile([128, 512], BF, tag="raw",
                                          name=f"{key}raw{n}")
                        nc.scalar.activation(raw[:], ps[:], IDN,
                                             bias=bqk_s[:, bidx:bidx + 1])
                        t1 = ropetmp.tile([128, 512], BF, tag="rt",
                                          name=f"{key}t1_{n}")
                        nc.vector.tensor_mul(t1[:], raw[:],
                                             ropecs_s[:, 0, tl:tl + 512])
                        sw = ropetmp.tile([128, 512], BF, tag="rt",
                                          name=f"{key}sw{n}")
                        nc.vector.stream_shuffle(sw[:], raw[:], SHUF_MASK)
                        nc.vector.tensor_mul(sw[:], sw[:],
                                             ropecs_s[:, 1, tl:tl + 512])
                        nc.vector.tensor_add(dst[:], t1[:], sw[:])
                    return f

                def vblock(m):
                    def f():
                        if m == 0:
                            st["vps"] = psP.tile([128, 512], F32, tag="proj",
                                                 name=f"vps{n}")
                        vps = st["vps"]
                        xt = st["xt"]
                        for kc in range(8):
                            nc.tensor.matmul(
                                vps[:, m * 128:(m + 1) * 128],
                                xt[:, kc, m * 128:(m + 1) * 128],
                                wqkv_s[:, 2, kc, :],
                                start=(m == 0 and kc == 0),
                                stop=(m == 3 and kc == 7),
                                skip_group_check=True)
                    return f

                def vpost():
                    vps = st.pop("vps")
                    src = vps[:].rearrange("p (m h d) -> p h m d", m=4, h=HPC)
                    dst = vaug[b][:, :, blk0:blk0 + 4, 0:64]
                    nc.vector.tensor_copy(dst, src)

                return ([xload, proj(0, "q"), proj(1, "k"),
                         post("q", 0, qrot[n]), post("k", 1, krot[n])]
                        + [vblock(m) for m in range(4)] + [vpost])

            # ---- phase B: attention for one (batch, 512-query-block) -----
            def b_steps(b, i, pe_tail=False):
                nj = 4 * (i + 1)
                tq = 4 * b + i
                st = {"ex": {}}

                def sc(j):
                    def f():
                        c0 = max(0, j - 4 * i) * 128
                        S = psS.tile([128, 1024], F32, tag="S",
                                     name=f"S{b}_{i}_{j}")
                        kchunk = 4 * b + j // 4
                        koff = (j % 4) * 128
                        for h in range(HPC):
                            klhs = krot[kchunk][h * 64:h * 64 + 64,
                                                koff:koff + 128]
                            qrhs = qrot[tq][h * 64:h * 64 + 64, c0:512]
                            nc.tensor.matmul(S[:, h * 512 + c0:(h + 1) * 512],
                                             klhs, qrhs, start=True, stop=True)
                        st[("S", j)] = S
                    return f

                def ex(j):
                    def f():
                        c0 = max(0, j - 4 * i) * 128
                        S = st.pop(("S", j))
                        e = expool.tile([128, HPC, 512], BF, tag="ex",
                                        name=f"ex{b}_{i}_{j}")
                        sv = S[:].rearrange("p (h t) -> p h t", h=HPC)
                        nc.scalar.activation(e[:, :, c0:], sv[:, :, c0:], EXP)
                        if j >= 4 * i:
                            for h in range(HPC):
                                nc.vector.tensor_mul(e[:, h, c0:c0 + 128],
                                                     e[:, h, c0:c0 + 128],
                                                     wotri_s[:, 8, :])
                        if use_pad_mask:
                            nc.vector.tensor_scalar_mul(
                                e[:, :, c0:], e[:, :, c0:],
                                pad_s[:, b * 16 + j:b * 16 + j + 1])
                        st["ex"][j] = e
                    return f

                def pv(j):
                    def f():
                        c0 = max(0, j - 4 * i) * 128
                        diag = j >= 4 * i
                        if j == 0:
                            st["cps"] = psC.tile([65, 1024], F32, tag="cps",
                                                 name=f"cps{b}_{i}")
                        cps = st["cps"]
                        for d in st.pop("defer", []):
                            d()
                        e = st["ex"].pop(j)
                        last = j == nj - 1
                        for h in range(HPC):
                            va = vaug[b][:, h, j, :]
                            if diag and c0 + 128 < 512 and not last:
                                # main (unmasked) range now; the masked
                                # 128-sub is deferred one j-step so it
                                # never waits on the DVE mask backlog
                                nc.tensor.matmul(
                                    cps[:, h * 512 + c0 + 128:
                                        (h + 1) * 512],
                                    va, e[:, h, c0 + 128:512],
                                    start=(j == 0), stop=False,
                                    skip_group_check=True)

                                def sub(h=h, e=e, va=va, c0=c0):
                                    nc.tensor.matmul(
                                        cps[:, h * 512 + c0:
                                            h * 512 + c0 + 128],
                                        va, e[:, h, c0:c0 + 128],
                                        start=False, stop=False,
                                        skip_group_check=True)
                                st.setdefault("defer", []).append(sub)
                            else:
                                nc.tensor.matmul(
                                    cps[:, h * 512 + c0:(h + 1) * 512],
                                    va, e[:, h, c0:512],
                                    start=(j == 0), stop=last,
                                    skip_group_check=True)
                    return f

                def tail_copy():
                    cps = st["cps"]
                    cu = normtmp.tile([65, 1024], BF, tag="cu",
                                      name=f"cu{b}_{i}")
                    _evict(CFG["cu_eng"], cu[:], cps[:])
                    st["cu"] = cu

                def tail_rec():
                    cu = st["cu"]
                    rec = normtmp.tile([1, 1024], BF, tag="rec",
                                       name=f"rec{b}_{i}")
                    with nc.allow_low_precision(reason="bf16 softmax denom"):
                        nc.vector.reciprocal(rec[:], cu[64:65, :])
                    scr = drampool.tile([1, 1024], BF, tag="scr",
                                        name=f"scr{b}_{i}")
                    nc.sync.dma_start(out=scr[:], in_=rec[:])
                    bc = normtmp.tile([64, 1024], BF, tag="bc",
                                      name=f"bc{b}_{i}")
                    nc.sync.dma_start(out=bc[:],
                                      in_=scr[:].partition_broadcast(64))
                    st["bc"] = bc

                def tail_mul():
                    cu, bc = st.pop("cu"), st.pop("bc")
                    st.pop("cps")
                    for h in range(HPC):
                        nc.vector.tensor_mul(
                            ctxt[tq][h * 64:(h + 1) * 64, :],
                            cu[0:64, h * 512:(h + 1) * 512],
                            bc[:, h * 512:(h + 1) * 512])

                def tail_pe():
                    # broadcast 1/denom with a PE ones-matmul: short
                    # dependency chain, no DMA round trip.  The broadcast
                    # bounces through SBUF so the normalize muls have only
                    # one PSUM operand (walrus TensorTensor constraint).
                    cps = st.pop("cps")
                    rec = normtmp.tile([1, 1024], BF, tag="rec",
                                       name=f"rec{b}_{i}")
                    bcp = psS.tile([64, 1024], F32, tag="S",
                                   name=f"bcp{b}_{i}")
                    for h in range(HPC):
                        # per-head reciprocal so the first broadcast
                        # matmul starts half a reciprocal earlier
                        with nc.allow_low_precision(
                                reason="bf16 softmax denom"):
                            nc.vector.reciprocal(
                                rec[:, h * 512:(h + 1) * 512],
                                cps[64:65, h * 512:(h + 1) * 512])
                        nc.tensor.matmul(bcp[:, h * 512:(h + 1) * 512],
                                         ones_s[:, 0:64],
                                         rec[:, h * 512:(h + 1) * 512],
                                         start=True, stop=True)
                    bcs = normtmp.tile([64, 1024], BF, tag="bc",
                                       name=f"bcs{b}_{i}")
                    nc.scalar.activation(bcs[:], bcp[:], CPY)
                    for h in range(HPC):
                        nc.vector.tensor_mul(
                            ctxt[tq][h * 64:(h + 1) * 64, :],
                            cps[0:64, h * 512:(h + 1) * 512],
                            bcs[:, h * 512:(h + 1) * 512])

                steps = [sc(0), ex(0)]
                for j in range(1, nj):
                    steps += [sc(j), ex(j), pv(j - 1)]
                steps.append(pv(nj - 1))
                if pe_tail or CFG.get("norm") == "pe":
                    steps.append(tail_pe)
                else:
                    steps += [tail_copy, tail_rec, tail_mul]
                return steps

            # ---- phase C: output projection for one 512-token chunk ------
            def _evict(eng, dst, src):
                if eng == "p":
                    nc.gpsimd.tensor_copy(dst, src)
                elif eng == "d":
                    nc.vector.tensor_copy(dst, src)
                else:
                    nc.scalar.activation(dst, src, CPY)

            def c_steps(n, evict=None):
                # PSUM->SBUF bf16 eviction on DVE/Act (Pool cannot read
                # PSUM, DMA cannot source from PSUM in this build)
                evict = evict or CFG["evict"]
                t0 = n * 512
                st = {}

                def ce(e):
                    def f():
                        if e == 0:
                            st["stage"] = outstage.tile([128, 8, 512], BF,
                                                        tag="os",
                                                        name=f"os{n}")
                        ps = psP.tile([128, 512], F32, tag="proj",
                                      name=f"op{n}_{e}")
                        nc.tensor.matmul(ps[:], wotri_s[:, e, :], ctxt[n][:],
                                         start=True, stop=True)
                        _evict(evict[e], st["stage"][:, e, :], ps[:])
                    return f

                def cstore():
                    # two half-stores: a full-chunk 2.9us transfer blocks
                    # latency-critical tiny DMAs behind it on the ring
                    stage = st.pop("stage")
                    nc.sync.dma_start(out=out8[:, 0:4, t0:t0 + 512],
                                      in_=stage[:, 0:4, :])
                    nc.sync.dma_start(out=out8[:, 4:8, t0:t0 + 512],
                                      in_=stage[:, 4:8, :])

                return [ce(e) for e in range(8)] + [cstore]

            def c_steps_tail(n, evict="adad"):
                # after the last attention phase the psS slots are free:
                # pair e-blocks into [128,1024] slots, one 2-block eviction
                # each on alternating engines, and store per pair so the
                # final DMA covers only the last quarter of the chunk
                t0 = n * 512
                st = {}

                def cep(ep):
                    def f():
                        if ep == 0:
                            st["stage"] = outstage.tile([128, 8, 512], BF,
                                                        tag="os",
                                                        name=f"os{n}")
                        sp = psS.tile([128, 1024], F32, tag="S",
                                      name=f"cS{n}_{ep}")
                        for half in range(2):
                            e = 2 * ep + half
                            nc.tensor.matmul(
                                sp[:, half * 512:(half + 1) * 512],
                                wotri_s[:, e, :], ctxt[n][:],
                                start=True, stop=True)
                        src = sp[:].rearrange("p (h t) -> p h t", h=2)
                        dst = st["stage"][:, 2 * ep:2 * ep + 2, :]
                        _evict(evict[ep], dst, src)
                        nc.sync.dma_start(
                            out=out8[:, 2 * ep:2 * ep + 2, t0:t0 + 512],
                            in_=dst)
                    return f

                return [cep(ep) for ep in range(4)]

            # ---- master schedule -----------------------------------------
            # Emission order IS the per-queue program order, and cross-
            # engine waits are only as precise as that order - so each
            # phase's PE work is emitted before any later DMA enters the
            # queues it depends on.
            emit_consts_a1()
            a0 = a_steps(0, split_load=True)
            a1 = a_steps(1)
            steps = [emit_consts_a1second,           # wq kc4-7
                     a0[0], a0[1],                   # xt0, qproj
                     emit_consts_a1b,                # wk/wv BEFORE kproj
                     a0[2],                          # kproj
                     emit_consts_a2, emit_vaug_init,
                     a1[0]] + a0[3:]                 # xt1 prefetch, a0 rest
            # A1's compute is the only PE work available while B00 waits on
            # A0's rope chain, so emit it first, then B00 un-woven
            steps += [a1[1], a1[2], emit_consts_a3] + a1[3:]
            steps.append(emit_consts_b)
            steps += b_steps(0, 0)
            if CFG["plan"] == "P1":
                plan = [
                    (b_steps(0, 1), a_steps(2)),
                    (b_steps(0, 2), a_steps(3)),
                    (b_steps(0, 3), a_steps(4)),
                    (b_steps(1, 0), a_steps(5)),
                    (b_steps(1, 1), a_steps(6) + c_steps(0)),
                    (b_steps(1, 2), a_steps(7) + c_steps(1)),
                    (b_steps(1, 3, pe_tail=True),
                     c_steps(2) + c_steps(3) + c_steps(4)
                     + c_steps(5, "adadadad") + c_steps(6, "adadadad")),
                ]
            elif CFG["plan"] == "P4":
                # C chunks spread as early as dependencies allow, late
                # chunks evicted DVE-only (Act is exp-bound there)
                el = CFG.get("evict_late", "dddddddd")
                plan = [
                    (b_steps(0, 1), a_steps(2)),
                    (b_steps(0, 2), a_steps(3) + c_steps(0)),
                    (b_steps(0, 3), a_steps(4) + c_steps(1)),
                    (b_steps(1, 0), a_steps(5) + c_steps(2, el)),
                    (b_steps(1, 1), a_steps(6) + c_steps(3, el)),
                    (b_steps(1, 2), a_steps(7) + c_steps(4, el)),
                    (b_steps(1, 3, pe_tail=True),
                     c_steps(5, el) + c_steps(6, el)),
                ]
            elif CFG["plan"] == "P2":
                plan = [
                    (b_steps(0, 1), a_steps(2)),
                    (b_steps(0, 2), a_steps(3) + c_steps(0)),
                    (b_steps(0, 3), a_steps(4) + c_steps(1)),
                    (b_steps(1, 0), a_steps(5) + c_steps(2)),
                    (b_steps(1, 1), a_steps(6) + c_steps(3)),
                    (b_steps(1, 2), a_steps(7) + c_steps(4)),
                    (b_steps(1, 3, pe_tail=True), c_steps(5) + c_steps(6)),
                ]
            else:  # P3
                plan = [
                    (b_steps(0, 1), a_steps(2)),
                    (b_steps(0, 2), a_steps(3)),
                    (b_steps(0, 3), a_steps(4) + c_steps(0)),
                    (b_steps(1, 0), a_steps(5) + c_steps(1)),
                    (b_steps(1, 1), a_steps(6) + c_steps(2)),
                    (b_steps(1, 2), a_steps(7) + c_steps(3)),
                    (b_steps(1, 3, pe_tail=True),
                     c_steps(4) + c_steps(5) + c_steps(6)),
                ]
            for idx, (b_, f_) in enumerate(plan):
                front = 0.9 if idx == len(plan) - 1 else 0.7
                steps += _weave(b_, f_, front)
            steps += c_steps_tail(7, "adad")
            for s in steps:
                s()
    return nc


_NC_CACHE = {}


def _get_nc(use_pad_mask: bool):
    key = use_pad_mask
    if key not in _NC_CACHE:
        _NC_CACHE[key] = _build_nc(use_pad_mask)
    return _NC_CACHE[key]


def _host_inputs(x, attention_mask, qkv_w, qkv_b, out_w, use_pad_mask):
    """Build the 8 per-core input maps (everything bf16 except biases)."""
    x = np.asarray(x, dtype=np.float32)
    qkv_w = np.asarray(qkv_w, dtype=np.float32)
    qkv_b = np.asarray(qkv_b, dtype=np.float32)
    out_w = np.asarray(out_w, dtype=np.float32)

    xT = x.reshape(NT, DIM).T                                  # [1024, 4096]
    x8 = np.ascontiguousarray(
        xT.reshape(8, 128, NT).transpose(1, 0, 2)).astype(NPBF)  # [128,8,NT]
    C, S = _rope_tables()
    tri = _tri_mask()
    perm = _qk_row_perm()

    in_maps = []
    for c in range(NCORES):
        heads = [HPC * c + h for h in range(HPC)]
        qrows = np.concatenate([h * Dh + perm for h in heads])
        vrows = np.concatenate([h * Dh + np.arange(Dh) for h in heads])
        wq_c = qkv_w[qrows, :] * SCALE                         # [128, 1024]
        wk_c = qkv_w[DIM + qrows, :]
        wv_c = qkv_w[2 * DIM + vrows, :]

        def pack_w(w_c):
            # [128, 8, 128]: (p, kc, m) = W[m, kc*128+p]
            return np.ascontiguousarray(
                w_c.T.reshape(8, 128, RPC).transpose(1, 0, 2)).astype(NPBF)

        wqkv = np.ascontiguousarray(np.stack(
            [pack_w(wq_c), pack_w(wk_c), pack_w(wv_c)], axis=1))
        bqk = np.ascontiguousarray(np.stack(
            [qkv_b[qrows] * SCALE, qkv_b[DIM + qrows]],
            axis=1).astype(np.float32))                        # [128, 2]
        ropecs = np.ascontiguousarray(
            np.stack([C, S], axis=1).astype(NPBF))             # [128, 2, T]
        # wo is used with ctx rows on partitions - no K-chunk repack
        wo_cT = out_w[:, c * RPC:(c + 1) * RPC].T              # [128, 1024]
        wotri = np.ascontiguousarray(np.concatenate(
            [wo_cT.reshape(RPC, 8, 128).astype(NPBF),
             tri.astype(NPBF)[:, None, :]], axis=1))           # [128, 9, 128]
        m = {
            "xT8": x8,
            "wqkv": wqkv,
            "bqk": bqk,
            "ropecs": ropecs,
            "wotri": wotri,
        }
        if use_pad_mask:
            pad = np.asarray(attention_mask, dtype=np.float32)  # [B, T]
            m["padv"] = np.ascontiguousarray(
                pad.reshape(B, T // 128, 128).transpose(0, 2, 1))
        in_maps.append(m)
    return in_maps


def kernel(x, attention_mask, qkv_w, qkv_b, out_w, out_b):
    from concourse.bass_utils import run_bass_kernel_spmd

    use_pad_mask = not np.asarray(attention_mask).all()
    nc = _get_nc(use_pad_mask)
    in_maps = _host_inputs(x, attention_mask, qkv_w, qkv_b, out_w,
                           use_pad_mask)
    res = run_bass_kernel_spmd(nc, in_maps, list(range(NCORES)))
    acc = np.zeros((128, 8, NT), dtype=np.float32)
    for c in range(NCORES):
        acc += np.asarray(res.results[c]["out8"]).astype(np.float32)
    outT = acc.transpose(1, 0, 2).reshape(DIM, NT)             # [1024, 4096]
    bv = np.asarray(qkv_b, dtype=np.float32)[2 * DIM:]
    extra = np.asarray(out_w, dtype=np.float32) @ bv
    out = outT.T + np.asarray(out_b, dtype=np.float32)[None, :] + extra[None, :]
    return np.ascontiguousarray(out.reshape(B, T, DIM), dtype=np.float32)

